# revision 2
# baseline (speedup 1.0000x reference)
"""DifferenceOfGaussiansFFT on 8 Trainium2 NeuronCores — v2.

Sharding: core c -> (batch b = c//4, quarter q = c%4).  Each core computes
dog planes [8q, 8q+8) for its batch plus one halo plane each side
(recompute, no collectives).  Uniform SPMD program: 11 S1/U slots, 10 dog
slots, 8 pools per core; out-of-range slots neutralized with scale=0 /
bias=-1e38 host data (so all cores run the identical instruction stream).

Math per core:
    S1_t = I @ A_t                   pass1 (horizontal blur)
    U_t  = A_t^T @ S1_t              pass2 (vertical blur)
    dog'_s = sigma_s*(U_s - U_{s+1}) - thn       (thn = nextafter(0.001))
    lm'  = maxpool3d(dog')  (separable F -> H -> W)
    mask = (Relu(lm') == dog')       [one fused scalar_tensor_tensor op]
    host adds thn back to lm'.

Matmuls run in bf16 hi/lo pairs, 3 terms per logical matmul
(Ah*Bh + Al*Bh + Ah*Bl), giving ~2^-17 relative weight error — 4x tighter
than the PE's native 2-pass fp32 (~2^-15) at 3/4 the streaming cost and
with working LDWEIGHTS batching (fp32 PE matmuls cost 4 cycles/row;
bf16 costs 1).  S1 is evicted as an hi/lo pair (cast + subtract).

Maxpool H direction (partition dim) goes through a DRAM round trip:
write tri + clamp rows into a [514,512] scratch, read back y-1/y+1
shifted copies.  These are HBM-path DMAs that spread across all 16 SDMA
engines — the v1 SBUF->SBUF partition-shift DMAs all pinned to ONE DMA
engine at 25 GB/s and serialized the whole kernel.
"""

import math

import numpy as np

_IMG = 512
_B = 2
_F = 33
_R = 51
_TH = 0.001
_NCORES = 8
_NS1 = 11   # S1/U slots per core
_ND = 10    # dog slots per core

# pass1 window packing: per kt a 256-wide column window of A
_JLO1 = [0, 51, 205, 256]
_OFF1 = [0, 256, 512, 768]
# emission order and start flags (kt0/kt3 first-writers, kt1/kt2 accumulate)
_W1SEQ = [(0, True), (3, True), (1, False), (2, False)]
# baseline-style pass1 segments: (kt, [(jlo, jhi), ...]) in kt order; the
# accumulation start flag is True exactly for the first matmul of the group
_SEGS1 = [(0, [(0, 179)]), (1, [(77, 179), (179, 307)]),
          (2, [(205, 307), (307, 435)]), (3, [(333, 435), (435, 512)])]

_cache = {}


def _thn():
    return float(np.nextafter(np.float32(_TH), np.float32(np.inf)))


def _build_host_data(kernels, sigmas):
    import ml_dtypes

    fp16 = np.float16
    kernels = np.asarray(kernels, dtype=np.float32)
    sigmas = np.asarray(sigmas, dtype=np.float32)
    F = kernels.shape[0]
    assert F == _F

    # exact 1D taps: kernel = outer(t, t) with t = row / sqrt(center)
    A32 = np.zeros((F, _IMG, _IMG), dtype=np.float32)
    idx = np.arange(_IMG)
    for f in range(F):
        k2 = kernels[f].astype(np.float64)
        taps = k2[_R, : 2 * _R + 1] / math.sqrt(k2[_R, _R])
        A = np.zeros((_IMG, _IMG), dtype=np.float64)
        for d in range(-_R, _R + 1):
            v = taps[_R + d]
            src = idx[max(0, -d): _IMG - max(0, d)]
            A[src, src + d] = v
        A32[f] = A.astype(np.float32)

    def pair(x):
        h = x.astype(fp16)
        l = (x - h.astype(np.float32)).astype(fp16)
        return np.ascontiguousarray(h), np.ascontiguousarray(l)

    # pass1 window storage: aw1[f, p, OFF1[kt]+c] = A[128kt+p, JLO1[kt]+c]
    aw1 = np.zeros((F, 128, 1024), dtype=np.float32)
    for f in range(F):
        for kt in range(4):
            rows = A32[f][128 * kt: 128 * kt + 128]
            aw1[f, :, _OFF1[kt]: _OFF1[kt] + 256] = rows[:, _JLO1[kt]: _JLO1[kt] + 256]
    aw1h, aw1l = pair(aw1 * np.float32(2.0 ** 15))

    # pass2 compact block storage: abp[f, kt, p, m] = A[128kt+p, 128(kt-1)+m]
    abp = np.zeros((F, 4, 128, 384), dtype=np.float32)
    for f in range(F):
        for kt in range(4):
            rows = A32[f][128 * kt: 128 * kt + 128]
            jlo = max(0, 128 * (kt - 1))
            jhi = min(_IMG, 128 * (kt + 2))
            abp[f, kt, :, jlo - 128 * (kt - 1): jhi - 128 * (kt - 1)] = rows[:, jlo:jhi]
    abph, abpl = pair(abp * np.float32(2.0 ** 15))

    return aw1h, aw1l, abph, abpl, sigmas, fp16


def _build_program():
    import concourse.bass as bass
    import concourse.mybir as mybir
    import concourse.tile as tile
    from concourse import bacc

    fp32 = mybir.dt.float32
    fp16 = mybir.dt.float16
    u8 = mybir.dt.uint8
    Alu = mybir.AluOpType
    Act = mybir.ActivationFunctionType

    nc = bacc.Bacc("TRN2", target_bir_lowering=False)

    Th_d = nc.dram_tensor("timgh", [_IMG, _IMG], fp16, kind="ExternalInput")
    Tl_d = nc.dram_tensor("timgl", [_IMG, _IMG], fp16, kind="ExternalInput")
    awh_d = nc.dram_tensor("aw1h", [_NS1, 128, 1024], fp16, kind="ExternalInput")
    awl_d = nc.dram_tensor("aw1l", [_NS1, 128, 1024], fp16, kind="ExternalInput")
    abh_d = nc.dram_tensor("abh", [_NS1, 4, 128, 384], fp16, kind="ExternalInput")
    abl_d = nc.dram_tensor("abl", [_NS1, 4, 128, 384], fp16, kind="ExternalInput")
    dsc_d = nc.dram_tensor("dsc", [128, _ND], fp32, kind="ExternalInput")
    dbi_d = nc.dram_tensor("dbi", [128, _ND], fp32, kind="ExternalInput")
    lm_d = nc.dram_tensor("lm", [8, _IMG, _IMG], fp32, kind="ExternalOutput")
    mk_d = nc.dram_tensor("mask", [8, _IMG, _IMG], u8, kind="ExternalOutput")

    with tile.TileContext(nc) as tc:
        with (
            tc.tile_pool(name="const", bufs=1) as constp,
            tc.tile_pool(name="aw", bufs=2) as awp,
            tc.tile_pool(name="s1", bufs=2) as s1p,
            tc.tile_pool(name="u", bufs=2) as up,
            tc.tile_pool(name="dog", bufs=3) as dogp,
            tc.tile_pool(name="q", bufs=1) as qp,
            tc.tile_pool(name="tri", bufs=1) as trip,
            tc.tile_pool(name="sh", bufs=3) as shp,
            tc.tile_pool(name="hx", bufs=2) as hxp,
            tc.tile_pool(name="scr", bufs=2) as scrp,
            tc.tile_pool(name="lmp", bufs=1) as lmp,
            tc.tile_pool(name="rl", bufs=1) as rlp,
            tc.tile_pool(name="msk", bufs=1) as mskp,
            tc.tile_pool(name="hs", bufs=3, space="DRAM") as hsp,
            tc.tile_pool(name="ps1", bufs=4, space="PSUM") as ps1p,
            tc.tile_pool(name="ps2", bufs=4, space="PSUM") as ps2p,
        ):
            Th_sb = constp.tile([128, 4, _IMG], fp16, tag="th")
            nc.sync.dma_start(Th_sb[:], Th_d.rearrange("(t p) y -> p t y", p=128))
            Tl_sb = constp.tile([128, 4, _IMG], fp16, tag="tl")
            nc.sync.dma_start(Tl_sb[:], Tl_d.rearrange("(t p) y -> p t y", p=128))
            dsc_sb = constp.tile([128, _ND], fp32, tag="dsc")
            nc.sync.dma_start(dsc_sb[:], dsc_d[:])
            dbi_sb = constp.tile([128, _ND], fp32, tag="dbi")
            nc.sync.dma_start(dbi_sb[:], dbi_d[:])
            ABH = {}
            ABL = {}

            S1H = {}
            S1L = {}
            U = {}
            DOG = {}
            Q = {}

            def make_s1(t):
                awh = awp.tile([128, 1024], fp16, tag="awh")
                nc.scalar.dma_start(awh[:], awh_d[t])
                awl = awp.tile([128, 1024], fp16, tag="awl")
                nc.scalar.dma_start(awl[:], awl_d[t])
                abh = awp.tile([128, 4, 384], fp16, tag="abh")
                nc.scalar.dma_start(abh[:], abh_d[t].rearrange("k p c -> p k c"))
                abl = awp.tile([128, 4, 384], fp16, tag="abl")
                nc.scalar.dma_start(abl[:], abl_d[t].rearrange("k p c -> p k c"))
                ABH[t] = abh
                ABL[t] = abl
                s1h = s1p.tile([128, 4, _IMG], fp16, tag="s1h")
                s1l = s1p.tile([128, 4, _IMG], fp16, tag="s1l")
                for mt in range(4):
                    ps = ps1p.tile([128, _IMG], fp32, tag="ps1")
                    nmm = 0
                    nt = 3 * sum(1 for _kt, _segs in _SEGS1 for _s in _segs)
                    for kt, segs in _SEGS1:
                        for (jlo, jhi) in segs:
                            o = _OFF1[kt] + jlo - _JLO1[kt]
                            w = jhi - jlo
                            terms = ((Th_sb, awh), (Tl_sb, awh), (Th_sb, awl))
                            for ti, (Tw, aw) in enumerate(terms):
                                nc.tensor.matmul(
                                    ps[:, jlo:jhi],
                                    Tw[:, kt, 128 * mt: 128 * mt + 128],
                                    aw[:, o: o + w],
                                    start=(nmm == 0),
                                    stop=(nmm == nt - 1),
                                )
                                nmm += 1
                    nc.scalar.activation(s1h[:, mt, :], ps[:], Act.Copy)
                    nc.vector.tensor_tensor(
                        s1l[:, mt, :], ps[:], s1h[:, mt, :], Alu.subtract)
                S1H[t] = s1h
                S1L[t] = s1l

            def make_u(t):
                abh = ABH[t]
                abl = ABL[t]
                u = up.tile([128, 4, _IMG], fp32, tag="u")
                for mt in range(4):
                    ps = ps2p.tile([128, _IMG], fp32, tag="ps2")
                    kts = [k for k in (mt - 1, mt, mt + 1) if 0 <= k < 4]
                    mms = []
                    for AB, S1 in ((abh, S1H), (abl, S1H), (abh, S1L)):
                        for kt in kts:
                            moff = 128 * (mt - kt + 1)
                            mms.append((AB[:, kt, moff: moff + 128],
                                        S1[t][:, kt, :]))
                    for i, (w, rhs) in enumerate(mms):
                        nc.tensor.matmul(ps[:], w, rhs, start=(i == 0),
                                         stop=(i == len(mms) - 1))
                    nc.scalar.activation(u[:, mt, :], ps[:], Act.Copy)
                U[t] = u
                if t - 2 in S1H:
                    del S1H[t - 2]
                    del S1L[t - 2]
                del ABH[t]
                del ABL[t]

            def make_dog(s):
                tmp = scrp.tile([128, 4, _IMG], fp32, tag="scr")
                d = dogp.tile([128, 4, _IMG], fp32, tag="dog")
                for mt in range(4):
                    nc.gpsimd.tensor_tensor(
                        tmp[:, mt, :], U[s][:, mt, :], U[s + 1][:, mt, :],
                        Alu.subtract)
                    nc.scalar.activation(
                        d[:, mt, :], tmp[:, mt, :], Act.Identity,
                        scale=dsc_sb[:, s: s + 1], bias=dbi_sb[:, s: s + 1],
                    )
                DOG[s] = d
                if s - 1 in U:
                    del U[s - 1]

            def make_q(x):
                qt = qp.tile([128, 4, _IMG], fp32, tag="q")
                for mt in range(4):
                    nc.vector.tensor_tensor(
                        qt[:, mt, :], DOG[x][:, mt, :], DOG[x + 1][:, mt, :],
                        Alu.max)
                Q[x] = qt

            def pool(sm):
                # plane m = J0 + sm ; output index sm-1
                tri = trip.tile([128, 4, _IMG], fp32, tag="tri")
                for mt in range(4):
                    nc.vector.tensor_tensor(
                        tri[:, mt, :], Q[sm - 1][:, mt, :],
                        DOG[sm + 1][:, mt, :], Alu.max)
                if sm - 2 in Q:
                    del Q[sm - 2]

                # H (y = 128t+p) 3-max via DRAM round trip with clamped edge rows
                hs = hsp.tile([514, _IMG], fp32, tag="hs")
                nc.sync.dma_start(
                    hs[1:257].rearrange("(t p) x -> p t x", p=128),
                    tri[:, 0:2, :])
                nc.scalar.dma_start(
                    hs[257:513].rearrange("(t p) x -> p t x", p=128),
                    tri[:, 2:4, :])
                nc.sync.dma_start(hs[0:1], tri[0:1, 0:1, :])
                nc.scalar.dma_start(hs[513:514], tri[127:128, 3:4, :])
                shA = shp.tile([128, 4, _IMG], fp32, tag="sh")
                nc.sync.dma_start(
                    shA[:, 0:2, :], hs[0:256].rearrange("(t p) x -> p t x", p=128))
                nc.scalar.dma_start(
                    shA[:, 2:4, :], hs[256:512].rearrange("(t p) x -> p t x", p=128))
                shB = shp.tile([128, 4, _IMG], fp32, tag="sh")
                nc.sync.dma_start(
                    shB[:, 0:2, :], hs[2:258].rearrange("(t p) x -> p t x", p=128))
                nc.scalar.dma_start(
                    shB[:, 2:4, :], hs[258:514].rearrange("(t p) x -> p t x", p=128))
                hA = hxp.tile([128, 4, _IMG], fp32, tag="hx")
                nc.vector.tensor_tensor(hA[:], tri[:], shA[:], Alu.max)
                hB = hxp.tile([128, 4, _IMG], fp32, tag="hx")
                nc.vector.tensor_tensor(hB[:], hA[:], shB[:], Alu.max)

                # W (x, free dim) 3-max
                wA = scrp.tile([128, 4, _IMG], fp32, tag="scr")
                nc.vector.tensor_tensor(
                    wA[:, :, 0:511], hB[:, :, 0:511], hB[:, :, 1:512], Alu.max)
                nc.vector.tensor_copy(wA[:, :, 511:512], hB[:, :, 511:512])
                lm = lmp.tile([128, 4, _IMG], fp32, tag="lm")
                nc.vector.tensor_tensor(
                    lm[:, :, 1:512], wA[:, :, 0:511], wA[:, :, 1:512], Alu.max)
                nc.vector.tensor_copy(lm[:, :, 0:1], wA[:, :, 0:1])
                nc.sync.dma_start(
                    lm_d[sm - 1].rearrange("(t p) x -> p t x", p=128), lm[:])

                # mask = (Relu(lm') == dog')
                rl = rlp.tile([128, 4, _IMG], fp32, tag="rl")
                nc.scalar.activation(rl[:], lm[:], Act.Relu)
                mk = mskp.tile([128, 4, _IMG], u8, tag="msk")
                nc.vector.tensor_tensor(mk[:], rl[:], DOG[sm][:], Alu.is_equal)
                nc.sync.dma_start(
                    mk_d[sm - 1].rearrange("(t p) x -> p t x", p=128), mk[:])
                if sm - 1 in DOG:
                    del DOG[sm - 1]

            for t in range(_NS1):
                make_s1(t)
                make_u(t)
                if t >= 1:
                    make_dog(t - 1)
                if t >= 2:
                    make_q(t - 2)
                if t >= 3:
                    pool(t - 2)

    nc.compile()
    return nc


def kernel(input, kernels, sigmas):
    import jax  # noqa: F401
    from concourse.bass_utils import run_bass_kernel_spmd

    input = np.asarray(input, dtype=np.float32)
    aw1h, aw1l, abph, abpl, sig, bf16 = _cache.setdefault(
        "host", _build_host_data(kernels, sigmas))

    if "prog" not in _cache:
        _cache["prog"] = _build_program()
    nc = _cache["prog"]

    thn = _thn()
    in_maps = []
    for c in range(_NCORES):
        b, qq = c // 4, c % 4
        J0 = 8 * qq - 1
        gs = [min(max(J0 + t, 0), _F - 1) for t in range(_NS1)]

        T = np.ascontiguousarray(np.transpose(input[b]))  # [x, y]
        Th = T.astype(bf16)  # bf16 name bound to np.float16 from host data
        Tl = (T - Th.astype(np.float32)).astype(bf16)

        dsc = np.zeros((128, _ND), dtype=np.float32)
        dbi = np.zeros((128, _ND), dtype=np.float32)
        for s in range(_ND):
            j = J0 + s
            if 0 <= j < _F - 1:
                dsc[:, s] = sig[j] * np.float32(2.0 ** -30)
                dbi[:, s] = -thn
            else:
                dsc[:, s] = 0.0
                dbi[:, s] = -1e38
        in_maps.append({
            "timgh": Th, "timgl": Tl,
            "aw1h": np.ascontiguousarray(aw1h[gs]),
            "aw1l": np.ascontiguousarray(aw1l[gs]),
            "abh": np.ascontiguousarray(abph[gs]),
            "abl": np.ascontiguousarray(abpl[gs]),
            "dsc": dsc, "dbi": dbi,
        })

    res = run_bass_kernel_spmd(
        nc, in_maps, core_ids=list(range(_NCORES)),
        trace=_cache.get("trace", False),
        tmpdir=_cache.get("tmpdir"),
    )
    _cache["last_res"] = res

    lm_full = np.empty((_B, _F - 1, _IMG, _IMG), dtype=np.float32)
    mk_full = np.empty((_B, _F - 1, _IMG, _IMG), dtype=bool)
    for c in range(_NCORES):
        b, qq = c // 4, c % 4
        lm_full[b, 8 * qq: 8 * qq + 8] = res.results[c]["lm"] + np.float32(thn)
        mk_full[b, 8 * qq: 8 * qq + 8] = res.results[c]["mask"] != 0
    return mk_full, lm_full



# revision 24
# speedup vs baseline: 1.0616x; 1.0616x over previous
"""DifferenceOfGaussiansFFT on 8 Trainium2 NeuronCores — v3.

Sharding: core c -> (batch b = c//4, quarter q = c%4).  Each core computes
dog planes [8q, 8q+8) for its batch plus one halo plane each side
(recompute, no collectives): 11 blur slots, 10 dogs, 8 pools per core.

Math per core (all layouts TRANSPOSED: partition = x = W, free = y = H):
    pass1: S1[y, x'] = sum_x T[x, y] A[x, x']     (stationary image blocks,
           moving banded A windows; per-slot window width 128+2R_t)
    pass2: U^T[x', y'] = sum_y S1[y, x'] A[y, y'] (stationary S1 blocks,
           SAME moving A windows — A is symmetric)
    W_t   = sigma_t*U_t - thn                (ACT eviction, scale+bias)
    dog_{t-1} = psB_t*(-sigma_{t-1}*2^-30) + W_{t-1}   (one STT from PSUM)
    lm'   = maxpool3d(dog')  (F via q/tri, H free-dim shifts, W via DRAM
            round trip for partition shifts)
    mask  = ((lm' max 0) is_equal dog')      (one fused STT, u8)
    host adds thn back and transposes [x,y] -> [y,x].

Matmuls are bf16-pair fp16 hi/lo, 3 terms (Ah*Bh + Al*Bh + Ah*Bl),
weights scaled 2^15 per pass (U comes out 2^30-scaled; compensated in the
ACT/STT scale tables).  PSUM start flag zeroes the whole bank, so only the
first matmul per bank sets start=True.
"""

import math

import numpy as np

_IMG = 512
_B = 2
_F = 33
_R = 51
_TH = 0.001
_NCORES = 8
_NS1 = 11   # blur slots per core
_ND = 10    # dog slots per core

# per-slot radius: slot t's worst filter over cores is q=3's f = min(23+t, 32)
def _slot_radii(sigmas):
    rads = [int(5.0 * float(s) + 0.5) for s in sigmas]
    return [rads[min(23 + t, _F - 1)] for t in range(_NS1)]


def _windows(Rt):
    # (jlo, width) per kt; coverage of band [128kt-R, 128kt+128+R) clipped
    return [
        (0, 128 + Rt),
        (128 - Rt, 128 + 2 * Rt),
        (256 - Rt, 128 + 2 * Rt),
        (384 - Rt, 128 + Rt),
    ]


_cache = {}


def _thn():
    return float(np.nextafter(np.float32(_TH), np.float32(np.inf)))


def _build_host_data(kernels, sigmas):
    fp16 = np.float16
    kernels = np.asarray(kernels, dtype=np.float32)
    sigmas = np.asarray(sigmas, dtype=np.float32)
    F = kernels.shape[0]
    assert F == _F

    # exact 1D taps: kernel = outer(t, t) with t = row / sqrt(center)
    A32 = np.zeros((F, _IMG, _IMG), dtype=np.float32)
    idx = np.arange(_IMG)
    for f in range(F):
        k2 = kernels[f].astype(np.float64)
        taps = k2[_R, : 2 * _R + 1] / math.sqrt(k2[_R, _R])
        A = np.zeros((_IMG, _IMG), dtype=np.float64)
        for d in range(-_R, _R + 1):
            v = taps[_R + d]
            src = idx[max(0, -d): _IMG - max(0, d)]
            A[src, src + d] = v
        A32[f] = A.astype(np.float32)

    radii = _slot_radii(sigmas)

    def pair(x):
        h = x.astype(fp16)
        l = (x - h.astype(np.float32)).astype(fp16)
        return np.ascontiguousarray(h), np.ascontiguousarray(l)

    # per-quarter banded window tensors [NS1, 128, 1024], fp16 pair, x 2^15
    aw_by_q = {}
    for qq in range(4):
        J0 = 8 * qq - 1
        gs = [min(max(J0 + t, 0), _F - 1) for t in range(_NS1)]
        aw = np.zeros((_NS1, 128, 1024), dtype=np.float32)
        for t in range(_NS1):
            f = gs[t]
            for kt, (jlo, w) in enumerate(_windows(radii[t])):
                rows = A32[f][128 * kt: 128 * kt + 128]
                aw[t, :, 256 * kt: 256 * kt + w] = rows[:, jlo: jlo + w]
        aw_by_q[qq] = pair(aw * np.float32(2.0 ** 15))

    return aw_by_q, sigmas, radii, fp16


def _build_program(radii):
    import concourse.bass as bass  # noqa: F401
    import concourse.mybir as mybir
    import concourse.tile as tile
    from concourse import bacc

    fp32 = mybir.dt.float32
    fp16 = mybir.dt.float16
    u8 = mybir.dt.uint8
    Alu = mybir.AluOpType
    Act = mybir.ActivationFunctionType

    nc = bacc.Bacc("TRN2", target_bir_lowering=False)

    Th_d = nc.dram_tensor("timgh", [_IMG, _IMG], fp16, kind="ExternalInput")
    Tl_d = nc.dram_tensor("timgl", [_IMG, _IMG], fp16, kind="ExternalInput")
    awh_d = nc.dram_tensor("awh", [_NS1, 128, 1024], fp16, kind="ExternalInput")
    awl_d = nc.dram_tensor("awl", [_NS1, 128, 1024], fp16, kind="ExternalInput")
    # scale/bias tables: [128, NS1] each
    wsc_d = nc.dram_tensor("wsc", [128, _NS1], fp32, kind="ExternalInput")
    wbi_d = nc.dram_tensor("wbi", [128, _NS1], fp32, kind="ExternalInput")
    dsc_d = nc.dram_tensor("dsc", [128, _NS1], fp32, kind="ExternalInput")
    lm_d = nc.dram_tensor("lm", [8, _IMG, _IMG], fp32, kind="ExternalOutput")
    mk_d = nc.dram_tensor("mask", [8, _IMG, _IMG], u8, kind="ExternalOutput")

    with tile.TileContext(nc) as tc:
        with (
            tc.tile_pool(name="const", bufs=1) as constp,
            tc.tile_pool(name="aw", bufs=2) as awp,
            tc.tile_pool(name="s1", bufs=2) as s1p,
            tc.tile_pool(name="wv", bufs=2) as wvp,
            tc.tile_pool(name="dv", bufs=2) as dvp,
            tc.tile_pool(name="dog", bufs=3) as dogp,
            tc.tile_pool(name="q", bufs=2) as qp,
            tc.tile_pool(name="tri", bufs=1) as trip,
            tc.tile_pool(name="m1", bufs=2) as m1p,
            tc.tile_pool(name="mh", bufs=2) as mhp,
            tc.tile_pool(name="sh", bufs=3) as shp,
            tc.tile_pool(name="hx", bufs=2) as hxp,
            tc.tile_pool(name="lmp", bufs=2) as lmp,
            tc.tile_pool(name="msk", bufs=2) as mskp,
            tc.tile_pool(name="hs", bufs=3, space="DRAM") as hsp,
            tc.tile_pool(name="psA", bufs=4, space="PSUM") as psAp,
            tc.tile_pool(name="psB", bufs=1, space="PSUM") as psBp,
        ):
            Th_sb = constp.tile([128, 4, _IMG], fp16, tag="th")
            nc.sync.dma_start(Th_sb[:], Th_d.rearrange("(t p) y -> p t y", p=128))
            Tl_sb = constp.tile([128, 4, _IMG], fp16, tag="tl")
            nc.sync.dma_start(Tl_sb[:], Tl_d.rearrange("(t p) y -> p t y", p=128))
            wsc_sb = constp.tile([128, _NS1], fp32, tag="wsc")
            nc.sync.dma_start(wsc_sb[:], wsc_d[:])
            wbi_sb = constp.tile([128, _NS1], fp32, tag="wbi")
            nc.sync.dma_start(wbi_sb[:], wbi_d[:])
            dsc_sb = constp.tile([128, _NS1], fp32, tag="dsc")
            nc.sync.dma_start(dsc_sb[:], dsc_d[:])

            AW = {}
            S1 = {}
            WV = {}   # W_t = sigma_t * U_t - thn
            PSB = {}
            DOG = {}
            Q = {}

            def load_aw(t):
                awh = awp.tile([128, 1024], fp16, tag="awh")
                nc.sync.dma_start(awh[:], awh_d[t])
                awl = awp.tile([128, 1024], fp16, tag="awl")
                nc.sync.dma_start(awl[:], awl_d[t])
                AW[t] = (awh, awl)

            def mt_matmuls(t, stat_h, stat_l, out_ap_fn):
                """12 matmuls per mt: 4 kt x 3 hi/lo terms into psum."""
                awh, awl = AW[t]
                wins = _windows(radii[t])
                for mt in range(4):
                    nmm = 0
                    for kt in (0, 3, 1, 2):
                        jlo, w = wins[kt]
                        terms = ((stat_h, awh), (stat_l, awh), (stat_h, awl))
                        for sb, aw in terms:
                            nc.tensor.matmul(
                                out_ap_fn(mt, jlo, w),
                                sb[:, kt, 128 * mt: 128 * mt + 128],
                                aw[:, 256 * kt: 256 * kt + w],
                                start=(nmm == 0),
                                stop=(nmm == 11),
                            )
                            nmm += 1

            def slot(t):
                # pass1: 4 separate 1-bank psum tiles for per-mt eviction
                # pipelining (PE can roll into pass2 while evictions drain)
                psA = [psAp.tile([128, _IMG], fp32, tag="ps", name=f"psA{t}_{m}")
                       for m in range(4)]
                mt_matmuls(t, Th_sb, Tl_sb,
                           lambda mt, jlo, w: psA[mt][:, jlo: jlo + w])
                s1h = s1p.tile([128, 4, _IMG], fp16, tag="s1h")
                s1l = s1p.tile([128, 4, _IMG], fp16, tag="s1l")
                for mt in range(4):
                    nc.scalar.activation(s1h[:, mt, :], psA[mt][:], Act.Copy)
                    nc.vector.tensor_tensor(
                        s1l[:, mt, :], psA[mt][:], s1h[:, mt, :], Alu.subtract)
                # (s1l stays on vector: gpsimd cannot read PSUM)
                # pass2: one 4-bank psum tile (consumers are not PE-critical)
                psB = psBp.tile([128, 4, _IMG], fp32, tag="psb")
                mt_matmuls(t, s1h, s1l,
                           lambda mt, jlo, w: psB[:, mt, jlo: jlo + w])
                S1[t] = (s1h, s1l)
                if t - 1 in S1:
                    del S1[t - 1]
                # W_t eviction (scalar): W = psB*wsc[t] + wbi[t]
                wv = wvp.tile([128, 4, _IMG], fp32, tag="wv")
                nc.scalar.activation(
                    wv[:], psB[:], Act.Identity,
                    scale=wsc_sb[:, t: t + 1], bias=wbi_sb[:, t: t + 1])
                WV[t] = wv
                # D_t = psB_t * dsc[t] (scalar ACT), then
                # dog_{t-1} = D_t + W_{t-1} on gpsimd (the only engine with
                # spare capacity; it cannot read PSUM or do max, but add is
                # in its supported op set)
                if t >= 1:
                    dsb = dvp.tile([128, 4, _IMG], fp32, tag="dv")
                    nc.scalar.activation(
                        dsb[:], psB[:], Act.Identity,
                        scale=dsc_sb[:, t: t + 1])
                    d = dogp.tile([128, 4, _IMG], fp32, tag="dog")
                    nc.gpsimd.tensor_tensor(
                        d[:], dsb[:], WV[t - 1][:], Alu.add)
                    DOG[t - 1] = d
                    del WV[t - 1]

            def make_q(x):
                # q_x = max(dog_x, dog_{x+1})   (max only exists on DVE)
                qt = qp.tile([128, 4, _IMG], fp32, tag="q")
                nc.vector.tensor_tensor(qt[:], DOG[x][:], DOG[x + 1][:], Alu.max)
                Q[x] = qt

            def pool(o):
                # output plane o (0..7): tri over dogs o, o+1, o+2
                tri = trip.tile([128, 4, _IMG], fp32, tag="tri")
                nc.vector.tensor_tensor(
                    tri[:], Q[o][:], DOG[o + 2][:], Alu.max)
                if o - 1 in Q:
                    del Q[o - 1]
                if o - 1 in DOG:
                    del DOG[o - 1]

                # H (y, free dim) 3-max
                m1 = m1p.tile([128, 4, _IMG], fp32, tag="m1")
                nc.vector.tensor_tensor(
                    m1[:, :, 0:511], tri[:, :, 0:511], tri[:, :, 1:512],
                    Alu.max)
                nc.vector.tensor_copy(m1[:, :, 511:512], tri[:, :, 511:512])
                mh = mhp.tile([128, 4, _IMG], fp32, tag="mh")
                nc.vector.tensor_tensor(
                    mh[:, :, 1:512], m1[:, :, 0:511], m1[:, :, 1:512], Alu.max)
                nc.vector.tensor_copy(mh[:, :, 0:1], m1[:, :, 0:1])

                # W (x = partition dim) 3-max via DRAM round trip, edge clamp
                hs = hsp.tile([514, _IMG], fp32, tag="hs")
                nc.sync.dma_start(
                    hs[1:513].rearrange("(m p) y -> p m y", p=128), mh[:])
                nc.scalar.dma_start(hs[0:1], mh[0:1, 0:1, :])
                nc.scalar.dma_start(hs[513:514], mh[127:128, 3:4, :])
                shA = shp.tile([128, 4, _IMG], fp32, tag="sh")
                nc.sync.dma_start(
                    shA[:], hs[0:512].rearrange("(m p) y -> p m y", p=128))
                shB = shp.tile([128, 4, _IMG], fp32, tag="sh")
                nc.sync.dma_start(
                    shB[:], hs[2:514].rearrange("(m p) y -> p m y", p=128))
                hA = hxp.tile([128, 4, _IMG], fp32, tag="hx")
                nc.vector.tensor_tensor(hA[:], mh[:], shA[:], Alu.max)
                lmT = lmp.tile([128, 4, _IMG], fp32, tag="lm")
                nc.vector.tensor_tensor(lmT[:], hA[:], shB[:], Alu.max)
                nc.sync.dma_start(
                    lm_d[o].rearrange("(m p) y -> p m y", p=128), lmT[:])

                # mask = ((lm' max 0) == dog'_{o+1})   (one fused STT)
                mk = mskp.tile([128, 4, _IMG], u8, tag="msk")
                nc.vector.scalar_tensor_tensor(
                    mk[:], lmT[:], 0.0, DOG[o + 1][:], Alu.max, Alu.is_equal)
                nc.sync.dma_start(
                    mk_d[o].rearrange("(m p) y -> p m y", p=128), mk[:])

            load_aw(0)
            for t in range(_NS1):
                if t + 1 < _NS1:
                    load_aw(t + 1)
                slot(t)
                if 2 <= t <= 9:
                    make_q(t - 2)
                if t >= 3:
                    pool(t - 3)

    nc.compile()
    return nc


def kernel(input, kernels, sigmas):
    from concourse.bass_utils import run_bass_kernel_spmd

    input = np.asarray(input, dtype=np.float32)
    aw_by_q, sig, radii, bf16 = _cache.setdefault(
        "host", _build_host_data(kernels, sigmas))

    if "prog" not in _cache:
        _cache["prog"] = _build_program(radii)
    nc = _cache["prog"]

    thn = _thn()
    in_maps = []
    for c in range(_NCORES):
        b, qq = c // 4, c % 4
        J0 = 8 * qq - 1

        T = np.ascontiguousarray(np.transpose(input[b]))  # [x, y]
        Th = T.astype(bf16)
        Tl = (T - Th.astype(np.float32)).astype(bf16)

        wsc = np.zeros((128, _NS1), dtype=np.float32)
        wbi = np.zeros((128, _NS1), dtype=np.float32)
        dsc = np.zeros((128, _NS1), dtype=np.float32)
        for t in range(_NS1):
            j = J0 + t
            if 0 <= j < _F - 1:
                wsc[:, t] = sig[j] * np.float32(2.0 ** -30)
                wbi[:, t] = -thn
            else:
                wsc[:, t] = 0.0
                wbi[:, t] = -1e38
            jd = J0 + t - 1
            if 0 <= jd < _F - 1:
                dsc[:, t] = -sig[jd] * np.float32(2.0 ** -30)
            else:
                dsc[:, t] = 0.0
        awh, awl = aw_by_q[qq]
        in_maps.append({
            "timgh": Th, "timgl": Tl,
            "awh": awh, "awl": awl,
            "wsc": wsc, "wbi": wbi, "dsc": dsc,
        })

    res = run_bass_kernel_spmd(
        nc, in_maps, core_ids=list(range(_NCORES)),
        trace=_cache.get("trace", False),
        tmpdir=_cache.get("tmpdir"),
    )
    _cache["last_res"] = res

    lm_full = np.empty((_B, _F - 1, _IMG, _IMG), dtype=np.float32)
    mk_full = np.empty((_B, _F - 1, _IMG, _IMG), dtype=bool)
    for c in range(_NCORES):
        b, qq = c // 4, c % 4
        lm_c = res.results[c]["lm"]      # [8, x, y]
        mk_c = res.results[c]["mask"]
        lm_full[b, 8 * qq: 8 * qq + 8] = (
            np.transpose(lm_c, (0, 2, 1)) + np.float32(thn))
        mk_full[b, 8 * qq: 8 * qq + 8] = np.transpose(mk_c, (0, 2, 1)) != 0
    return mk_full, lm_full
